# revision 22
# baseline (speedup 1.0000x reference)
"""Trainium2 Bass kernel for the fused L2-embed / RMS-norm / tanh-gate module.

  sumsq[n,c] = sum_{h,w} x[n,c,h,w]^2
  embed      = sqrt(sumsq + eps) * alpha
  inv[n]     = rsqrt(mean_c(embed^2) + eps)
  z          = embed * gamma * inv + beta
  out        = x * (1 + tanh(z))

Data-parallel over the batch axis: 8 samples per NeuronCore, 8 cores.
HBM-bound; the 2e-2 rel-err budget admits bf16 transport (~2.4e-3), so x
ships as bf16 both ways: 25.7MB/core, ~59us at the ~435GB/s aggregate
DMA cap — that streaming time plus preamble/exit is the whole roofline.

Schedule (v2): keep the DMA saturated end to end.
- Loads: whole-sample halves alternate the two HWDGE rings (sync/
  scalar), 4 items per ring queued up front, L4-L7 fired from
  iterations 0-3.
- Squares (stage A): ACT does k0 of every sample plus k1 of s4-s6
  (11 halves x 2.9us); DVE does k1 of s0-s3 and s7 via fused
  square+accum (5 x 3.4us).  s7's k1 deliberately rides DVE so the
  last sample's sumsq never waits behind ACT's backlog.
- Stage B batched per 4-sample quad on DVE (2 quads): one PE matmul
  broadcasts the channel sum; rsqrt(m) and sqrt(u+eps) use the bit-
  trick seed + one Newton step (no ACT table, no DVE reciprocal);
  tanh on ACT.  ~23 small DVE ops per quad instead of ~13 per sample.
- Gate muls: all 16 halves on DVE (bf16 tensor_scalar, 4x mode,
  1.03us each), ordered s4,s5 before s6,s7 in quad 2.
- Stores: halves s0-s5 ride gpsimd's SWDGE queue as soon as each mul
  lands (~29us onward, draining while loads finish); s6/s7 split
  across both HWDGE rings at the end when those engines are idle,
  so the exit drain resolves on fast HWDGE sems.
"""

import json

import numpy as np

N, C, H, W = 64, 256, 56, 56
HW = H * W                    # 3136
NCORES = 8
NPC = N // NCORES             # samples per core
EPS = 1e-5
P = 128
K = C // P                    # free-dim channel halves per partition (2)
RSQRT_MAGIC = 0x5F3759DF
Q = 4                         # samples per stage-B quad

_cache = {}


# --------------------------------------------------------------------------
# BIR post-processing: the walrus build in this container allows at most one
# sync wait and one sync update per instruction.  Hoist excess waits onto
# NoOps inserted before the instruction (same engine/block); move excess
# updates of non-DMA instructions onto a NoOp right after.
# --------------------------------------------------------------------------
_nop_counter = [0]


def _mk_nop(engine, waits, updates, debug=0):
    _nop_counter[0] += 1
    return {
        "name": f"I-wsplit-{_nop_counter[0]}",
        "opcode": "NoOp",
        "engine": engine,
        "ins": [],
        "outs": [],
        "debug": debug,
        "sync_info": {"on_wait": waits, "on_update": updates},
    }


def _split_sync_waits(bir_json_bytes):
    d = json.loads(bir_json_bytes)
    for f in d.get("functions", []):
        for blk in f.get("blocks", []):
            new_insts = []
            for inst in blk.get("instructions", []):
                si = inst.get("sync_info")
                after = []
                if si:
                    waits = list(si.get("on_wait") or [])
                    updates = list(si.get("on_update") or [])
                    eng = inst.get("engine")
                    dbg = inst.get("debug", 0)
                    if len(waits) > 1:
                        for w in waits[:-1]:
                            new_insts.append(_mk_nop(eng, [w], [], dbg))
                        waits = waits[-1:]
                    if len(updates) > 1:
                        op = inst.get("opcode", "")
                        if "DMA" in op:
                            raise RuntimeError(
                                f"DMA instruction {inst.get('name')} has "
                                f"{len(updates)} sync updates; cannot split"
                            )
                        for u in updates[1:]:
                            after.append(_mk_nop(eng, [], [u], dbg))
                        updates = updates[:1]
                    si["on_wait"] = waits
                    si["on_update"] = updates
                new_insts.append(inst)
                new_insts.extend(after)
            blk["instructions"] = new_insts
    return json.dumps(d).encode()


def _patch_bass(nc):
    orig = nc.to_json_bytes

    def fixed(*a, **kw):
        return _split_sync_waits(orig(*a, **kw))

    nc.to_json_bytes = fixed
    return nc


# --------------------------------------------------------------------------
# Kernel build
# --------------------------------------------------------------------------
def _build():
    import concourse.bass as bass
    import concourse.tile as tile
    from concourse import mybir
    from concourse.tile import ScopedClock

    f32 = mybir.dt.float32
    bf16 = mybir.dt.bfloat16
    u32 = mybir.dt.uint32
    Alu = mybir.AluOpType
    Act = mybir.ActivationFunctionType

    class LeanExitTileContext(tile.TileContext):
        """Standard exit minus the second all-engine barrier (~3.4us).
        NRT only starts a subsequent execution after every engine stream has
        ended, and the sem clears sit on gpsimd's own stream, so the final
        barrier adds no ordering we need."""

        def _drain_and_barrier(self, tick_clock, wait_clock):
            drain_inst = self.nc.sync.drain()
            wait_clock.add_sem_waits(
                drain_inst.ins, ScopedClock({None: tick_clock.global_clock})
            )
            self.nc.all_engine_barrier()
            assert self.sems is not None
            popped = self.nc._tile_sem_poison_stack.pop()
            assert popped is self._sem_poison
            self.nc.clear_and_free_semaphores(
                list(self.sems.allocated().values())
            )

    nc = bass.Bass(trn_type="TRN2")
    x = nc.dram_tensor("x", [NPC, C, HW], bf16, kind="ExternalInput")
    # Host-precomputed param tile (see _run): [:, 0] = alpha^2/C
    # replicated per sample, [:, 1] = beta replicated, [:, 2, 0] =
    # alpha*gamma.  One 192B-per-partition SWDGE transfer replaces three
    # serialized 128-tiny-descriptor param DMAs (~3.5us each) plus all
    # the on-chip replication ops that were stalling the DVE queue.
    pp = nc.dram_tensor("pp", [P, 3, NPC, K], f32, kind="ExternalInput")
    out = nc.dram_tensor("out", [NPC, C, HW], bf16, kind="ExternalOutput")

    with LeanExitTileContext(nc) as tc:
        with (
            tc.tile_pool(name="xpool", bufs=1) as xpool,
            tc.tile_pool(name="scratch", bufs=1) as scratch,
            tc.tile_pool(name="small", bufs=4) as small,
            tc.tile_pool(name="singles", bufs=1) as singles,
            tc.tile_pool(name="ps", bufs=2, space="PSUM") as ps,
        ):
            # ---- one-time constants ----
            # channel c lives at (partition c//K, free-half c%K).
            ppt = singles.tile([P, 3, NPC, K], f32)
            nc.gpsimd.dma_start(out=ppt[:], in_=pp[:])
            a2q = ppt[:, 0]                          # alpha^2/C, [P,8,K]
            b4 = ppt[:, 1]                           # beta,      [P,8,K]
            ag_col = ppt[:, 2, 0]                    # alpha*gamma, [P,K]

            zero_bias = singles.tile([P, 1], f32)  # memset, not const-DMA:
            nc.vector.memset(zero_bias[:], 0.0)    # keeps ACT off the const
            # tensor DMA dependency that otherwise delays the first square

            # Dummy 1-element activation: pulls the ~1.3us ACT function
            # table load into the DMA preamble instead of paying for it
            # between the load triggers and the first real square.
            act_warm = singles.tile([P, 1], f32)
            nc.scalar.activation(
                out=act_warm[:], in_=zero_bias[:], func=Act.Square,
                bias=zero_bias[:, 0:1],
            )

            ones_t = singles.tile([P, P], f32)       # all-ones lhsT for col-sum
            nc.vector.memset(ones_t[:], 1.0)
            magicq = singles.tile([P, 5], u32)       # rsqrt seed [P,5]
            nc.vector.memset(magicq[:], RSQRT_MAGIC)
            magic8 = singles.tile([P, 5, K], u32)    # rsqrt seed [P,5K]
            nc.vector.memset(magic8[:], RSQRT_MAGIC)

            # ---- DMA plan (see module docstring) ----
            ring = (nc.sync, nc.scalar)
            xts, outs = [], []
            for n in range(NPC):
                xts.append(xpool.tile([P, K, HW], bf16, name=f"xt{n}"))
                outs.append(out[n].rearrange("(p a) hw -> p a hw", p=P))

            def load(n):
                # halves split across both rings: sample lands in one
                # half-transfer time, and 4 items/ring fit the queue depth
                xr = x[n].rearrange("(p a) hw -> p a hw", p=P)
                for k in range(K):
                    ring[(n + k) % 2].dma_start(out=xts[n][:, k], in_=xr[:, k])

            def store_half(n, k):
                if n >= NPC - 2:
                    # tail stores split across BOTH rings: by now the ring
                    # engines are done with loads/squares, the 3.2MB tail
                    # streams in parallel, and HWDGE completion sems
                    # resolve fast at the exit drain (SWDGE sems park ~8us)
                    ring[(n + k) % 2].dma_start(
                        out=outs[n][:, k], in_=xts[n][:, k]
                    )
                else:
                    nc.gpsimd.dma_start(out=outs[n][:, k], in_=xts[n][:, k])

            for n in range(Q):
                load(n)

            sq_act = scratch.tile([P, K, HW], bf16)   # ACT square dummy out
            sq_dve = scratch.tile([P, HW], bf16)      # DVE fused-square dummy

            # Uneven stage-B groups: group 0 = s0..s4 (its 10 gate-mul
            # halves keep the SWDGE store queue fed 40-59us, exactly the
            # window where a 4-sample group left it dry), group 1 =
            # s5..s7 (short tail).  k1-square engine assignment: DVE for
            # s0-s4 (ACT is busy with the early k0s) and s7 (so the last
            # sample never queues behind ACT); ACT absorbs s5/s6.
            GROUPS = [list(range(5)), list(range(5, NPC))]
            GRP_OF = {n: g for g, ns in enumerate(GROUPS) for n in ns}
            DVE_K1 = {0, 1, 2, 3, 4, 7}

            u4s = {g: small.tile([P, len(ns), K], f32, name=f"u4_{g}")
                   for g, ns in enumerate(GROUPS)}

            # Quad-2 squares take their unit-scalar / zero-bias operands
            # from tiles derived from quad 1's gate tile.  This is a pure
            # scheduling device: the tile scheduler's internal DMA model
            # thinks the late loads complete early, so without a real data
            # dependency it pins quad-2 squares (which wait on loads)
            # ahead of quad 1's stage B and gate muls in the sem-enforced
            # stream — observed as ~10us of dead DMA time mid-kernel.
            gate_dep = {}

            def squares(n):
                g = GRP_OF[n]
                u4 = u4s[g]
                i = n - GROUPS[g][0]
                xt = xts[n]
                one_s = 1.0 if g == 0 else gate_dep["one"][:, 0:1]
                zero_b = zero_bias if g == 0 else gate_dep["zero"]
                if n in DVE_K1:
                    nc.vector.scalar_tensor_tensor(
                        out=sq_dve[:],
                        in0=xt[:, 1],
                        scalar=one_s,
                        in1=xt[:, 1],
                        op0=Alu.mult,
                        op1=Alu.mult,
                        accum_out=u4[:, i, 1:2],
                    )
                    ks = (0,)
                else:
                    ks = (0, 1)
                for k in ks:
                    nc.scalar.activation(
                        out=sq_act[:, k],
                        in_=xt[:, k],
                        func=Act.Square,
                        bias=zero_b[:, 0:1],
                        accum_out=u4[:, i, k : k + 1],
                    )

            def stage_b_and_muls(q):
                """Stage B for group q + gate muls."""
                ns = GROUPS[q]
                M = len(ns)
                lo = ns[0]
                u4 = u4s[q]
                # ua = (u+eps) * alpha^2/C   -> mean contribution per chan
                ua = small.tile([P, M, K], f32, name=f"ua{q}")
                nc.vector.scalar_tensor_tensor(
                    out=ua[:], in0=u4[:], scalar=EPS,
                    in1=a2q[:, lo : lo + M],
                    op0=Alu.add, op1=Alu.mult,
                )
                # cross-partition channel sum, broadcast to all partitions
                cs = ps.tile([P, M, K], f32, name=f"cs{q}")
                nc.tensor.matmul(cs[:], ones_t[:], ua[:], start=True, stop=True)
                # m = sum_c + eps  (PSUM allows only one tensor input per op)
                mr = small.tile([P, M, 1], f32, name=f"mr{q}")
                nc.vector.tensor_reduce(
                    mr[:], cs[:], axis=mybir.AxisListType.X, op=Alu.add
                )
                m4 = small.tile([P, M], f32, name=f"m4{q}")
                nc.vector.tensor_scalar(
                    m4[:], mr[:, :, 0], EPS, None, op0=Alu.add
                )
                # y = rsqrt(m): bit-trick seed + 1 Newton step
                y4 = small.tile([P, M], f32, name=f"y4{q}")
                sh = small.tile([P, M], u32, name=f"sh{q}")
                nc.vector.tensor_scalar(
                    sh[:], m4[:].bitcast(u32), 1, None,
                    op0=Alu.logical_shift_right,
                )
                nc.vector.tensor_tensor(
                    out=y4[:].bitcast(u32), in0=magicq[:, :M], in1=sh[:],
                    op=Alu.subtract,
                )
                t4 = small.tile([P, M], f32, name=f"t4{q}")
                nc.vector.tensor_mul(t4[:], m4[:], y4[:])
                nc.vector.tensor_mul(t4[:], t4[:], y4[:])
                nc.vector.tensor_scalar(
                    t4[:], t4[:], -0.5, 1.5, op0=Alu.mult, op1=Alu.add
                )
                nc.vector.tensor_mul(y4[:], y4[:], t4[:])

                # s = sqrt(u+eps) = e * rsqrt(e), same bit trick
                e8 = small.tile([P, M, K], f32, name=f"e8{q}")
                nc.vector.tensor_scalar(e8[:], u4[:], EPS, None, op0=Alu.add)
                sh8 = small.tile([P, M, K], u32, name=f"sh8{q}")
                nc.vector.tensor_scalar(
                    sh8[:], e8[:].bitcast(u32), 1, None,
                    op0=Alu.logical_shift_right,
                )
                r8 = small.tile([P, M, K], f32, name=f"r8{q}")
                nc.vector.tensor_tensor(
                    out=r8[:].bitcast(u32), in0=magic8[:, :M], in1=sh8[:],
                    op=Alu.subtract,
                )
                t8 = small.tile([P, M, K], f32, name=f"t8{q}")
                nc.vector.tensor_mul(t8[:], e8[:], r8[:])
                nc.vector.tensor_mul(t8[:], t8[:], r8[:])
                nc.vector.tensor_scalar(
                    t8[:], t8[:], -0.5, 1.5, op0=Alu.mult, op1=Alu.add
                )
                nc.vector.tensor_mul(r8[:], r8[:], t8[:])
                s8 = small.tile([P, M, K], f32, name=f"s8{q}")
                nc.vector.tensor_mul(s8[:], e8[:], r8[:])

                # z = alpha*gamma * s * y + beta
                zq = small.tile([P, M, K], f32, name=f"zq{q}")
                for i in range(M):
                    nc.vector.scalar_tensor_tensor(
                        out=zq[:, i], in0=s8[:, i], scalar=y4[:, i : i + 1],
                        in1=ag_col[:], op0=Alu.mult, op1=Alu.mult,
                    )
                nc.vector.tensor_add(zq[:], zq[:], b4[:, lo : lo + M])

                # gate = 1 + tanh(z)  (tanh on ACT, +1 on DVE)
                gt = small.tile([P, M, K], f32, name=f"gt{q}")
                nc.scalar.activation(
                    out=gt[:], in_=zq[:], func=Act.Tanh, bias=zero_bias[:, 0:1]
                )
                nc.vector.tensor_scalar(gt[:], gt[:], 1.0, None, op0=Alu.add)

                if q == 0:
                    # scheduling anchors for quad-2 squares (see above):
                    # one = gt*0+1, zero = gt*0 — exact constants with a
                    # data dependency on this quad's finished gate.
                    one_g = singles.tile([P, 1], f32)
                    nc.vector.tensor_scalar(
                        one_g[:], gt[:, 0, 0:1], 0.0, 1.0,
                        op0=Alu.mult, op1=Alu.add,
                    )
                    zero_g = singles.tile([P, 1], f32)
                    nc.vector.tensor_scalar(
                        zero_g[:], gt[:, 0, 0:1], 0.0, None, op0=Alu.mult
                    )
                    gate_dep["one"] = one_g
                    gate_dep["zero"] = zero_g

                # gate muls + stores.  Quad 2: s4,s5 first so their SWDGE
                # stores start draining before the ring-store tail, and
                # the k1 halves of s6/s7 multiply on ACT (Copy with per-
                # partition scale, 2.9us/half, no function table) — ACT
                # is idle after the last tanh and this halves the DVE
                # mul tail, issuing the final ring stores ~2us earlier.
                for i in range(M):
                    n = ns[i]
                    for k in range(K):
                        if q == 1 and n >= NPC - 2 and k == 1:
                            nc.scalar.activation(
                                out=xts[n][:, k], in_=xts[n][:, k],
                                func=Act.Copy, bias=0.0,
                                scale=gt[:, i, k : k + 1],
                            )
                        else:
                            nc.vector.tensor_scalar_mul(
                                xts[n][:, k], in0=xts[n][:, k],
                                scalar1=gt[:, i, k : k + 1],
                            )
                        store_half(n, k)

            for n in range(NPC):
                squares(n)
                if n < Q:
                    load(n + Q)
                if n == GROUPS[0][-1]:
                    # high_priority: the list scheduler otherwise prefers
                    # later squares over this group's gate muls, stalling
                    # the store stream (observed: B1 deferred past s7's
                    # load, 10us of dead DMA).  Priority only breaks ties
                    # among ready instructions, so data deps still hold.
                    with tc.high_priority():
                        stage_b_and_muls(0)
            with tc.high_priority():
                stage_b_and_muls(1)

    return _patch_bass(nc)


def _get_nc():
    if "nc" not in _cache:
        _cache["nc"] = _build()
    return _cache["nc"]


def _ensure_axon_hooks_stub():
    """bass_utils imports antenv.axon_hooks when tracing is requested (e.g.
    via a stray BASS_TRACE=1); this image lacks that module. Provide a stub
    whose hook getter returns None so the untraced fallback path runs."""
    import sys
    import types

    try:
        import antenv.axon_hooks  # noqa: F401
    except ImportError:
        mod = types.ModuleType("antenv.axon_hooks")
        _holder = [None]
        mod.set_axon_ntff_profile_hook = lambda h: _holder.__setitem__(0, h)
        mod.get_axon_ntff_profile_hook = lambda: _holder[0]
        sys.modules["antenv.axon_hooks"] = mod


def _run(x, alpha, gamma, beta, trace=False, **spmd_kwargs):
    import ml_dtypes

    from concourse.bass_utils import run_bass_kernel_spmd

    _ensure_axon_hooks_stub()

    nc = _get_nc()
    bf16 = ml_dtypes.bfloat16
    x = np.asarray(x).reshape(N, C, HW).astype(bf16)
    alpha = np.asarray(alpha, dtype=np.float64)
    gamma = np.asarray(gamma, dtype=np.float64)
    beta = np.asarray(beta, dtype=np.float64)
    # Pre-bake the tiny [C] params into the on-chip layout (channel c at
    # partition c//K, half c%K), quad-replicated so stage B needs zero
    # on-chip setup ops: [:,0]=alpha^2/C, [:,1]=beta, [:,2,0]=alpha*gamma.
    a2 = (alpha * alpha / C).reshape(P, K)
    ag = (alpha * gamma).reshape(P, K)
    bt = beta.reshape(P, K)
    pp = np.zeros((P, 3, NPC, K), dtype=np.float32)
    pp[:, 0] = a2[:, None, :]
    pp[:, 1] = bt[:, None, :]
    pp[:, 2, 0] = ag
    pp = np.ascontiguousarray(pp)
    in_maps = [
        {
            "x": np.ascontiguousarray(x[c * NPC : (c + 1) * NPC]),
            "pp": pp,
        }
        for c in range(NCORES)
    ]
    res = run_bass_kernel_spmd(
        nc, in_maps, core_ids=list(range(NCORES)), trace=trace, **spmd_kwargs
    )
    full = np.concatenate([r["out"] for r in res.results], axis=0)
    return full.reshape(N, C, H, W).astype(np.float32), res


def kernel(x, alpha, gamma, beta):
    out, _ = _run(x, alpha, gamma, beta)
    return out
